# revision 9
# baseline (speedup 1.0000x reference)
"""Trainium2 Bass kernel for a 2-layer GATv2 encoder (nn_GATv2Encoder).

Strategy (8 NeuronCores, SPMD):
- Host sorts edges by dst; each core owns a contiguous 6400-dst-node range,
  so segment softmax/scatter are core-local. Within a core, dst nodes form
  256-node blocks; each block's edges are split by src parity into two
  sections, and each section's edges are grouped into four 64-node dst
  windows with a fixed (data-derived) tile count TW per window.
- Node-feature tables are bf16, node-pair-packed ([25600, 128]: row r =
  [f(2r)|f(2r+1)]), so int16 gather indices cover all 51200 padded nodes
  (idx = node>>1, parity selects the 64-column half). fd tables are local
  bf16 [6400, 128] rows.
- Layer-1 projections: every core computes the FULL graph GEMM from a
  per-core node-rotated bf16 x input (own nodes first) -- no AllGather at
  all; bias is added with a 1-row ones matmul into the same psum group.
  Layer-2 projections are data-parallel with a bf16 packed-fs AllGather.
- Edge phase per block: one merged dma_gather each for fs/fd (both
  sections), bf16 z-path (z = fs+fd, |z| on ACT, attn mult, head-reduce)
  running mostly in DVE 2x mode, exp on ACT. Scatter-add is a banded
  one-hot matmul: the one-hot is window-relative ([128, 64, T]) and built
  n-major in one bf16 2x DVE op; per-window psum accumulation groups open/
  close contiguously in a [72, 256] bank (num rows 0:64, den rows 64:72).
- Node features use an f-major ("v-order") column layout so the vals
  multiply broadcasts along the last (head) dim and stays in DVE 2x mode;
  host permutes all weights/consts and un-permutes the final output. The
  per-head logit reduce is an in-place pairwise bf16 tree of 2x adds.
- Softmax normalization folds into num*(1/den); BatchNorm stats use a tiny
  AllGather + local sum (cheaper than AllReduce), with padded-row
  corrections precomputed on host.
"""

import os
import numpy as np

# ---------------------------------------------------------------- constants
N_REAL = 50000
E_REAL = 800000
IN_DIM, HID, OUT_DIM = 128, 64, 64
H1, F1 = 8, 8
H2, F2 = 4, 16
SLOPE = 0.2
EPS = 1e-5
DEN_EPS = 1e-30

NCORES = 8
BLK = 256                      # nodes per block (one-hot / psum free width)
NB = 25                        # blocks per core
CORE_NODES = BLK * NB          # 6400
NPAD = NCORES * CORE_NODES     # 51200
HALF = NPAD // 2               # 25600 (int16-safe gather split)
TILE = 128                     # edges per matmul tile
WIN = 64                       # scatter one-hot band window (nodes)
NW = BLK // WIN                # windows per block (4)
# TW (tiles per window), SEC_T, CAP etc. are data-dependent; host_prep sets
# them via _set_geometry before build_program runs.
TW = None
SEC_T = None
CAP = None
T_BLK = None
SEC_COLS = None
BLK_COLS = None


def _set_geometry(tw):
    global TW, SEC_T, CAP, T_BLK, SEC_COLS, BLK_COLS
    TW = tw
    SEC_T = NW * tw
    CAP = SEC_T * TILE
    T_BLK = 2 * SEC_T
    SEC_COLS = CAP // 16
    BLK_COLS = 2 * SEC_COLS


# ---------------------------------------------------------------- host prep
def _wrap16(idx_list, cols):
    """int16 indices in dma_gather layout: [128, cols]; idx j at
    (partition j%16, col j//16), replicated across the 8 Q7 core groups."""
    flat = np.zeros(16 * cols, np.int16)
    flat[: len(idx_list)] = np.asarray(idx_list, np.int16)
    a = np.ascontiguousarray(flat.reshape(cols, 16).T)  # idx j -> (j%16, j//16)
    return np.tile(a, (8, 1))


def host_prep(src, dst):
    src = np.asarray(src).astype(np.int64)
    dst = np.asarray(dst).astype(np.int64)
    order = np.argsort(dst, kind="stable")
    s_src, s_dst = src[order], dst[order]
    counts = np.bincount(s_dst, minlength=NPAD)
    assert counts[:N_REAL].min() > 0, "zero in-degree node"
    cum = np.concatenate([[0], np.cumsum(counts)])

    # pass 1: per (core, block, parity, window) edge lists + max tile count
    secs = {}
    tw = 1
    for c in range(NCORES):
        base = c * CORE_NODES
        for b in range(NB):
            nb0 = base + b * BLK
            e0, e1 = cum[nb0], cum[min(nb0 + BLK, NPAD)]
            bs, bd = s_src[e0:e1], s_dst[e0:e1]
            rp = (bs - base) % NPAD          # rotated src position
            for sec in range(2):
                sel = (rp % 2) == sec
                ss, sg, dd = rp[sel], bs[sel], bd[sel] - nb0
                for w in range(NW):
                    m = (dd // WIN) == w
                    secs[(c, b, sec, w)] = (ss[m], sg[m], dd[m])
                    tw = max(tw, (len(ss[m]) + TILE - 1) // TILE)
    _set_geometry(tw)

    cores = []
    for c in range(NCORES):
        base = c * CORE_NODES
        fs_cols, fs2_cols, fd_cols = [], [], []
        dloc = np.full((128, NB * T_BLK), -1.0, np.float32)
        for b in range(NB):
            nb0 = base + b * BLK
            for sec in range(2):
                fsl = np.zeros(CAP, np.int64)
                fs2l = np.zeros(CAP, np.int64)
                fdl = np.zeros(CAP, np.int64)
                dl = np.full(CAP, -1.0, np.float32)
                for w in range(NW):
                    ss, sg, dd = secs[(c, b, sec, w)]
                    o = w * TW * TILE
                    fsl[o:o + len(ss)] = ss >> 1
                    fs2l[o:o + len(sg)] = sg >> 1
                    fdl[o:o + len(dd)] = dd + nb0 - base
                    dl[o:o + len(dd)] = dd - w * WIN
                fs_cols.append(_wrap16(fsl, SEC_COLS))
                fs2_cols.append(_wrap16(fs2l, SEC_COLS))
                fd_cols.append(_wrap16(fdl, SEC_COLS))
                dcol = (b * 2 + sec) * SEC_T
                dloc[:, dcol:dcol + SEC_T] = dl.reshape(SEC_T, 128).T
        cores.append(dict(
            fsidx=np.concatenate(fs_cols, 1),
            fsidx2=np.concatenate(fs2_cols, 1),
            fdidx=np.concatenate(fd_cols, 1),
            dloc=dloc,
        ))
    return cores


def _elu_np(x):
    return np.where(x > 0, x, np.exp(np.minimum(x, 0)) - 1).astype(np.float32)


def make_inputs(inputs):
    """Build the 8 per-core input maps for the bass program."""
    f32 = np.float32
    x = np.asarray(inputs["x"], f32)
    cores = host_prep(inputs["src"], inputs["dst"])

    xp = np.zeros((NPAD, IN_DIM), f32)
    xp[:N_REAL] = x

    w1 = np.concatenate([np.asarray(inputs["Wsrc1"], f32),
                         np.asarray(inputs["Wdst1"], f32)], 1)   # [128,128]
    w2 = np.concatenate([np.asarray(inputs["Wsrc2"], f32),
                         np.asarray(inputs["Wdst2"], f32)], 1)   # [64,128]
    b1 = np.concatenate([np.asarray(inputs["bsrc1"], f32),
                         np.asarray(inputs["bdst1"], f32)])      # [128]
    b2 = np.concatenate([np.asarray(inputs["bsrc2"], f32),
                         np.asarray(inputs["bdst2"], f32)])
    b1full = np.tile(b1[None, :], (128, 1)).astype(f32)
    b2full = np.tile(b2[None, :], (128, 1)).astype(f32)

    # lrelu(z) = (1+s)/2*z + (1-s)/2*|z|; the (1+s)/2 factor is folded into
    # the attention constants, the |z| path uses ACT Abs with scale (1-s)/(1+s)
    import ml_dtypes
    bf16 = ml_dtypes.bfloat16
    lr_a = (1.0 + SLOPE) / 2.0
    attn1f = np.tile(np.asarray(inputs["attn1"], f32).reshape(1, -1) * lr_a,
                     (128, 1)).astype(bf16)
    attn2f = np.tile(np.asarray(inputs["attn2"], f32).reshape(1, -1) * lr_a,
                     (128, 1)).astype(bf16)
    # n-major iota over the band window: iota[p, n*T_BLK + t] = n
    iota = np.tile(np.repeat(np.arange(WIN, dtype=f32), T_BLK)[None, :],
                   (128, 1)).astype(bf16)
    w1b = np.concatenate([np.asarray(inputs["Wsrc1"], f32),
                          np.asarray(inputs["Wdst1"], f32)], 1).astype(bf16)
    ones1 = np.ones((1, 128), bf16)
    b1row = b1[None, :].astype(bf16)

    # v-order (f-major) feature layout: v-col v holds std feature
    # cperm[v] = (v%H)*F + v//H; enables DVE 2x on the vals multiply.
    cperm1 = np.array([(v % H1) * F1 + v // H1 for v in range(HID)])
    cperm2 = np.array([(v % H2) * F2 + v // H2 for v in range(OUT_DIM)])
    pp1 = np.r_[cperm1, HID + cperm1]
    pp2 = np.r_[cperm2, OUT_DIM + cperm2]
    w1b = w1b[:, pp1]
    b1row = b1row[:, pp1]
    w2 = np.ascontiguousarray(w2[cperm1][:, pp2])
    b2full = np.ascontiguousarray(b2full[:, pp2])
    attn1f = np.ascontiguousarray(attn1f[:, cperm1])
    attn2f = np.ascontiguousarray(attn2f[:, cperm2])
    r1 = np.zeros((H1, HID), f32)
    r1[np.arange(HID) % H1, np.arange(HID)] = 1.0
    r2 = np.zeros((H2, OUT_DIM), f32)
    r2[np.arange(OUT_DIM) % H2, np.arange(OUT_DIM)] = 1.0

    npad_rows = NPAD - N_REAL
    bias1 = np.asarray(inputs["bias1"], f32)
    bias2 = np.asarray(inputs["bias2"], f32)
    cpad1 = _elu_np(bias1)
    bn1 = np.zeros((HID, 8), f32)
    bn1[:, 0] = bias1
    bn1[:, 1] = np.asarray(inputs["gamma1"], f32)
    bn1[:, 2] = np.asarray(inputs["beta1"], f32)
    bn1[:, 3] = npad_rows * cpad1
    bn1[:, 4] = npad_rows * cpad1 ** 2
    bn1[:, 5] = EPS
    bn2 = np.zeros((OUT_DIM, 8), f32)
    bn2[:, 0] = bias2
    bn2[:, 1] = np.asarray(inputs["gamma2"], f32)
    bn2[:, 2] = np.asarray(inputs["beta2"], f32)
    bn2[:, 3] = npad_rows * bias2
    bn2[:, 4] = npad_rows * bias2 ** 2
    bn2[:, 5] = EPS
    bn1 = np.ascontiguousarray(bn1[cperm1])
    bn2 = np.ascontiguousarray(bn2[cperm2])

    in_maps = []
    for c in range(NCORES):
        base = c * CORE_NODES
        xrot = np.roll(xp, -base, axis=0)  # my nodes first (rotated order)
        in_maps.append({
            "xT": np.ascontiguousarray(xrot.T).astype(bf16),
            "w1b": w1b, "ones1": ones1, "b1row": b1row,
            "w2": w2, "b2full": b2full,
            "attn1f": attn1f, "attn2f": attn2f,
            "iota": iota, "r1": r1, "r2": r2, "bn1": bn1, "bn2": bn2,
            "fsidx": cores[c]["fsidx"], "fsidx2": cores[c]["fsidx2"],
            "fdidx": cores[c]["fdidx"],
            "dloc": cores[c]["dloc"].astype(bf16),
        })
    return in_maps


# ---------------------------------------------------------------- bass program
def build_program():
    import concourse.bacc as bacc
    import concourse.tile as tile
    from concourse import mybir

    f32 = mybir.dt.float32
    bf16 = mybir.dt.bfloat16
    i16 = mybir.dt.int16
    Alu = mybir.AluOpType
    Act = mybir.ActivationFunctionType

    nc = bacc.Bacc(None, target_bir_lowering=False, num_devices=NCORES)
    RG = [list(range(NCORES))]

    # ---- I/O ----
    xT_d = nc.dram_tensor("xT", [IN_DIM, NPAD], bf16, kind="ExternalInput")
    w1b_d = nc.dram_tensor("w1b", [IN_DIM, 128], bf16, kind="ExternalInput")
    ones1_d = nc.dram_tensor("ones1", [1, 128], bf16, kind="ExternalInput")
    b1row_d = nc.dram_tensor("b1row", [1, 128], bf16, kind="ExternalInput")
    w2_d = nc.dram_tensor("w2", [HID, 128], f32, kind="ExternalInput")
    b2f_d = nc.dram_tensor("b2full", [128, 128], f32, kind="ExternalInput")
    a1_d = nc.dram_tensor("attn1f", [128, HID], bf16, kind="ExternalInput")
    a2_d = nc.dram_tensor("attn2f", [128, OUT_DIM], bf16, kind="ExternalInput")
    iota_d = nc.dram_tensor("iota", [128, WIN * T_BLK], bf16,
                            kind="ExternalInput")
    r1_d = nc.dram_tensor("r1", [H1, HID], f32, kind="ExternalInput")
    r2_d = nc.dram_tensor("r2", [H2, OUT_DIM], f32, kind="ExternalInput")
    bn1_d = nc.dram_tensor("bn1", [HID, 8], f32, kind="ExternalInput")
    bn2_d = nc.dram_tensor("bn2", [OUT_DIM, 8], f32, kind="ExternalInput")
    fsidx_d = nc.dram_tensor("fsidx", [128, NB * BLK_COLS], i16, kind="ExternalInput")
    fsidx2_d = nc.dram_tensor("fsidx2", [128, NB * BLK_COLS], i16, kind="ExternalInput")
    fdidx_d = nc.dram_tensor("fdidx", [128, NB * BLK_COLS], i16, kind="ExternalInput")
    dloc_d = nc.dram_tensor("dloc", [128, NB * T_BLK], bf16,
                            kind="ExternalInput")
    out_d = nc.dram_tensor("outT", [OUT_DIM, CORE_NODES], f32, kind="ExternalOutput")

    # ---- internal DRAM ----
    # bf16 node-pair-packed fs tables: row r = [fs(2r) | fs(2r+1)]
    # (layer 1 computed fully on-core in rotated order; layer 2 AllGathered)
    fs1p_full = nc.dram_tensor("fs1p_full", [NPAD // 2, 128], bf16)
    fs2p_loc = nc.dram_tensor("fs2p_loc", [CORE_NODES // 2, 128], bf16)
    fs2p_full = nc.dram_tensor("fs2p_full", [NPAD // 2, 128], bf16,
                               addr_space="Shared")
    # bf16 fd tables, 256B rows [fd(n) | junk]
    fd1p = nc.dram_tensor("fd1p", [CORE_NODES, 128], bf16)
    fd2p = nc.dram_tensor("fd2p", [CORE_NODES, 128], bf16)
    bnin = [nc.dram_tensor(f"bnin{i}", [64, 2], f32) for i in (1, 2)]
    bnout = [nc.dram_tensor(f"bnout{i}", [64 * NCORES, 2], f32,
                            addr_space="Shared") for i in (1, 2)]

    with tile.TileContext(nc) as tc:
        with (
            tc.tile_pool(name="const", bufs=1) as cpool,
            tc.tile_pool(name="gath", bufs=2) as gpool,
            tc.tile_pool(name="work", bufs=2) as wpool,
            tc.tile_pool(name="small", bufs=2) as spool,
            tc.tile_pool(name="node", bufs=1) as npool,
            tc.tile_pool(name="psA", bufs=2, space="PSUM") as psA,
            tc.tile_pool(name="psB", bufs=2, space="PSUM") as psB,
            tc.tile_pool(name="psG", bufs=2, space="PSUM") as psG,
        ):
            # ---- load constants ----
            def load(dram, shape, dtype=f32, pool=cpool):
                t = pool.tile(shape, dtype, tag=f"c_{dram.name}")
                nc.sync.dma_start(out=t[:], in_=dram[:, :])
                return t

            w1b_s = load(w1b_d, [IN_DIM, 128], bf16)
            ones1_s = load(ones1_d, [1, 128], bf16)
            b1row_s = load(b1row_d, [1, 128], bf16)
            w2_s = load(w2_d, [HID, 128])
            b2f_s = load(b2f_d, [128, 128])
            a1_s = load(a1_d, [128, HID], bf16)
            a2_s = load(a2_d, [128, OUT_DIM], bf16)
            iota_s = load(iota_d, [128, WIN * T_BLK], bf16)
            r1_s = load(r1_d, [H1, HID])
            r2_s = load(r2_d, [H2, OUT_DIM])
            bn1_s = load(bn1_d, [HID, 8])
            bn2_s = load(bn2_d, [OUT_DIM, 8])
            fsidx_s = load(fsidx_d, [128, NB * BLK_COLS], i16)
            fsidx2_s = load(fsidx2_d, [128, NB * BLK_COLS], i16)
            fdidx_s = load(fdidx_d, [128, NB * BLK_COLS], i16)
            dloc_s = load(dloc_d, [128, NB * T_BLK], bf16)

            h1_s = npool.tile([HID, CORE_NODES], f32, tag="h1")

            NT = CORE_NODES // 128  # node tiles per core for GEMMs

            def gemm_layer(get_lhs, K, w_s, bfull_s, fdp, pack_loc,
                           pack_full):
                # batched 4 node-tiles per psum bank
                t0 = 0
                while t0 < NT:
                    nt = min(4, NT - t0)
                    ps = psG.tile([128, 512], f32)
                    for k in range(nt):
                        sl = slice(k * 128, (k + 1) * 128)
                        nc.tensor.matmul(
                            out=ps[:, sl], lhsT=get_lhs(t0 + k),
                            rhs=w_s[:K, :], start=True, stop=True)
                    ps4 = ps[:].rearrange("p (t c) -> p t c", c=128)
                    gs = spool.tile([128, 512], f32, tag="gemm")
                    gs4 = gs[:].rearrange("p (t c) -> p t c", c=128)
                    nc.vector.tensor_tensor(
                        out=gs4[:, 0:nt, :], in0=ps4[:, 0:nt, :],
                        in1=bfull_s[:, None, :]
                            .to_broadcast([128, nt, 128]),
                        op=Alu.add)
                    # pack fs half to bf16 pair rows: row r=[fs(2r)|fs(2r+1)]
                    gp = spool.tile([128, 4 * 64], bf16, tag="gpack")
                    gp3 = gp[:].rearrange("p (t e) -> p t e", e=64)
                    nc.vector.tensor_copy(out=gp3[:, 0:nt, :],
                                          in_=gs4[:, 0:nt, 0:64])
                    nc.sync.dma_start(
                        out=pack_loc[t0 * 64:(t0 + nt) * 64, :]
                            .rearrange("(t r) (q e) -> (r q) t e", t=nt, q=2),
                        in_=gp3[:, 0:nt, :])
                    gpd = spool.tile([128, 4 * 64], bf16, tag="gpackd")
                    gpd3 = gpd[:].rearrange("p (t e) -> p t e", e=64)
                    nc.vector.tensor_copy(out=gpd3[:, 0:nt, :],
                                          in_=gs4[:, 0:nt, 64:128])
                    nc.sync.dma_start(
                        out=fdp[t0 * 128:(t0 + nt) * 128, 0:64]
                            .rearrange("(t r) e -> r t e", t=nt),
                        in_=gpd3[:, 0:nt, :])
                    t0 += nt
                nc.gpsimd.collective_compute(
                    "AllGather", Alu.bypass, replica_groups=RG,
                    ins=[pack_loc.ap().opt()], outs=[pack_full.ap().opt()])

            def gemm1_replicated():
                # full-graph GEMM on every core (rotated node order): no
                # collective needed; bias added via a 1-row ones matmul.
                # Batched 4 node-tiles per psum bank to amortize DMA/DVE.
                for t4 in range(NPAD // 512):
                    xg = spool.tile([IN_DIM, 512], bf16, tag="xg")
                    nc.sync.dma_start(
                        out=xg[:], in_=xT_d[:, t4 * 512:(t4 + 1) * 512])
                    ps = psG.tile([128, 512], f32)
                    for k in range(4):
                        sl = slice(k * 128, (k + 1) * 128)
                        nc.tensor.matmul(out=ps[:, sl], lhsT=xg[:, sl],
                                         rhs=w1b_s[:], start=True, stop=False)
                        nc.tensor.matmul(out=ps[:, sl], lhsT=ones1_s[:],
                                         rhs=b1row_s[:], start=False,
                                         stop=True)
                    ps4 = ps[:].rearrange("p (t c) -> p t c", c=128)
                    gp = spool.tile([128, 4 * 64], bf16, tag="gpack")
                    nc.vector.tensor_copy(
                        out=gp[:].rearrange("p (t e) -> p t e", e=64),
                        in_=ps4[:, :, 0:64])
                    nc.sync.dma_start(
                        out=fs1p_full[t4 * 256:(t4 + 1) * 256, :]
                            .rearrange("(t r) (q e) -> (r q) t e", t=4, q=2),
                        in_=gp[:].rearrange("p (t e) -> p t e", e=64))
                    nloc = min(max(CORE_NODES - t4 * 512, 0), 512) // 128
                    if nloc > 0:  # local-node fd table rows (may be partial)
                        gpd = spool.tile([128, 4 * 64], bf16, tag="gpackd")
                        nc.vector.tensor_copy(
                            out=gpd[:].rearrange("p (t e) -> p t e", e=64),
                            in_=ps4[:, :, 64:128])
                        nc.sync.dma_start(
                            out=fd1p[t4 * 512:t4 * 512 + nloc * 128, 0:64]
                                .rearrange("(t r) e -> r t e", t=nloc),
                            in_=gpd[:].rearrange("p (t e) -> p t e",
                                                 e=64)[:, 0:nloc, :])

            def edge_layer(Hh, Ff, pack_full, fdp, fsix_s, attn_s, rX_s,
                           bn_s, hout_s, do_elu):
                HF = Hh * Ff
                VW = Hh + HF  # vals width per tile
                for b in range(NB):
                    cA = b * BLK_COLS
                    fs_e = gpool.tile([128, T_BLK * 128], bf16, tag="fs")
                    fd_e = gpool.tile([128, T_BLK * 128], bf16, tag="fd")
                    nc.gpsimd.dma_gather(
                        out_ap=fs_e[:].rearrange("p (t e) -> p t e", e=128),
                        in_ap=pack_full[:, :],
                        idxs_ap=fsix_s[:, cA:cA + 2 * SEC_COLS],
                        num_idxs=2 * CAP, num_idxs_reg=2 * CAP,
                        elem_size=128, elem_step=128, single_packet=False)
                    nc.gpsimd.dma_gather(
                        out_ap=fd_e[:].rearrange("p (t e) -> p t e", e=128),
                        in_ap=fdp[:, :],
                        idxs_ap=fdidx_s[:, cA:cA + 2 * SEC_COLS],
                        num_idxs=2 * CAP, num_idxs_reg=2 * CAP,
                        elem_size=128, elem_step=128, single_packet=False)
                    fs3 = fs_e[:].rearrange("p (t e) -> p t e", e=128)
                    fd3 = fd_e[:].rearrange("p (t e) -> p t e", e=128)

                    # banded one-hot (bf16, n-major): window-relative
                    dcol = b * T_BLK
                    O_t = wpool.tile([128, WIN * T_BLK], bf16, tag="O")
                    O3 = O_t[:].rearrange("p (n t) -> p n t", t=T_BLK)
                    nc.vector.tensor_tensor(
                        out=O3,
                        in0=dloc_s[:, None, dcol:dcol + T_BLK]
                            .to_broadcast([128, WIN, T_BLK]),
                        in1=iota_s[:].rearrange("p (n t) -> p n t", t=T_BLK),
                        op=Alu.is_equal)

                    # z = fs + fd ; lrelu(z)*attn = (z + c|z|) * attn06
                    z_t = wpool.tile([128, T_BLK * 64], bf16, tag="z")
                    wz_t = wpool.tile([128, T_BLK * 64], bf16, tag="wz")
                    z3 = z_t[:].rearrange("p (t e) -> p t e", e=64)
                    for sec in range(2):
                        ts = slice(sec * SEC_T, (sec + 1) * SEC_T)
                        nc.vector.tensor_tensor(
                            out=z3[:, ts, :],
                            in0=fs3[:, ts, 64 * sec:64 * sec + 64],
                            in1=fd3[:, ts, 0:64], op=Alu.add)
                    nc.scalar.activation(
                        out=wz_t[:], in_=z_t[:], func=Act.Abs,
                        scale=(1.0 - SLOPE) / (1.0 + SLOPE))
                    nc.vector.tensor_tensor(
                        out=wz_t[:], in0=z_t[:], in1=wz_t[:], op=Alu.add)
                    nc.vector.tensor_tensor(
                        out=z3,
                        in0=wz_t[:].rearrange("p (t e) -> p t e", e=64),
                        in1=attn_s[:, None, :]
                            .to_broadcast([128, T_BLK, 64]),
                        op=Alu.mult)
                    # l = sum_f wz ; p = exp(l) into vals
                    # l[e,h] = sum_f zl[e,(f,h)] via in-place pairwise tree
                    # (bf16 2x adds beat the 1x tensor_reduce)
                    zv = z_t[:].rearrange("p (t v) -> p t v", v=64)
                    fw = Ff
                    while fw > 1:
                        half = (fw // 2) * Hh
                        nc.vector.tensor_tensor(
                            out=zv[:, :, 0:half], in0=zv[:, :, 0:half],
                            in1=zv[:, :, half:2 * half], op=Alu.add)
                        fw //= 2
                    vals = wpool.tile([128, T_BLK * VW], bf16, tag="vals")
                    vals3 = vals[:].rearrange("p (t v) -> p t v", v=VW)
                    nc.scalar.activation(
                        out=vals3[:, :, HF:VW],
                        in_=zv[:, :, 0:Hh],
                        func=Act.Exp)
                    for sec in range(2):
                        ts = slice(sec * SEC_T, (sec + 1) * SEC_T)
                        nc.vector.tensor_tensor(
                            out=vals3[:, ts, 0:HF].rearrange(
                                "p t (f h) -> p t f h", h=Hh),
                            in0=fs3[:, ts, 64 * sec:64 * sec + 64].rearrange(
                                "p t (f h) -> p t f h", h=Hh),
                            in1=vals3[:, ts, HF:VW][:, :, None, :]
                                .to_broadcast([128, SEC_T, Ff, Hh]),
                            op=Alu.mult)

                    # windowed scatter, window-major (contiguous psum groups)
                    ps_s = psA.tile([VW, BLK], f32, tag="scat")
                    for w in range(NW):
                        for sec in range(2):
                            for tt in range(TW):
                                t = sec * SEC_T + w * TW + tt
                                nc.tensor.matmul(
                                    out=ps_s[:, w * WIN:(w + 1) * WIN],
                                    lhsT=vals[:, t * VW:(t + 1) * VW],
                                    rhs=O3[:, :, t],
                                    start=(sec == 0 and tt == 0),
                                    stop=(sec == 1 and tt == TW - 1))

                    # normalize: out = num * (1/den) + bias
                    den = spool.tile([Hh, BLK], f32, tag="den")
                    nc.vector.tensor_scalar(
                        out=den[:], in0=ps_s[HF:VW, :], scalar1=DEN_EPS,
                        scalar2=None, op0=Alu.add)
                    rcp = spool.tile([Hh, BLK], f32, tag="rcp")
                    nc.vector.reciprocal(out=rcp[:], in_=den[:])
                    ps_r = psB.tile([HF, BLK], f32, tag="rrep")
                    nc.tensor.matmul(out=ps_r[:], lhsT=rX_s[:], rhs=rcp[:],
                                     start=True, stop=True)
                    rr = spool.tile([HF, BLK], f32, tag="rr")
                    nc.vector.tensor_copy(out=rr[:], in_=ps_r[:])
                    o1 = spool.tile([HF, BLK], f32, tag="o1")
                    nc.vector.tensor_tensor(
                        out=o1[:], in0=ps_s[0:HF, :], in1=rr[:], op=Alu.mult)
                    nsl = slice(b * BLK, (b + 1) * BLK)
                    if do_elu:
                        ob = spool.tile([HF, BLK], f32, tag="ob")
                        nc.vector.tensor_scalar(
                            out=ob[:], in0=o1[:], scalar1=bn_s[:, 0:1],
                            scalar2=None, op0=Alu.add)
                        m_t = spool.tile([HF, BLK], f32, tag="elum")
                        nc.vector.tensor_scalar(
                            out=m_t[:], in0=ob[:], scalar1=0.0,
                            scalar2=None, op0=Alu.min)
                        e_t = spool.tile([HF, BLK], f32, tag="elue")
                        nc.scalar.activation(out=e_t[:], in_=m_t[:],
                                             func=Act.Exp)
                        nc.vector.tensor_scalar(
                            out=m_t[:], in0=ob[:], scalar1=0.0,
                            scalar2=None, op0=Alu.max)
                        t_t = spool.tile([HF, BLK], f32, tag="elut")
                        nc.vector.tensor_tensor(
                            out=t_t[:], in0=e_t[:], in1=m_t[:], op=Alu.add)
                        nc.vector.tensor_scalar(
                            out=hout_s[:, nsl], in0=t_t[:], scalar1=-1.0,
                            scalar2=None, op0=Alu.add)
                    else:
                        nc.vector.tensor_scalar(
                            out=hout_s[:, nsl], in0=o1[:], scalar1=bn_s[:, 0:1],
                            scalar2=None, op0=Alu.add)

            def bn_norm(hin_s, bn_s, bnin_d, bnout_d, D):
                """BN stats (blockwise) + AllReduce; returns (scale, shift)."""
                NBB = (NB + 1) // 2
                s_cols = spool.tile([D, NBB], f32, tag="bnscols")
                q_cols = spool.tile([D, NBB], f32, tag="bnqcols")
                for i, b in enumerate(range(0, NB, 2)):
                    nb = min(2, NB - b)
                    nsl = slice(b * BLK, (b + nb) * BLK)
                    nc.vector.tensor_reduce(
                        out=s_cols[:, i:i + 1], in_=hin_s[:, nsl],
                        axis=mybir.AxisListType.X, op=Alu.add)
                    scr = spool.tile([D, 2 * BLK], f32, tag="nrmh")
                    nc.scalar.activation(
                        out=scr[:, 0:nb * BLK], in_=hin_s[:, nsl],
                        func=Act.Square, accum_out=q_cols[:, i:i + 1])
                st = spool.tile([D, 2], f32, tag="bnst")
                nc.vector.tensor_reduce(out=st[:, 0:1], in_=s_cols[:],
                                        axis=mybir.AxisListType.X, op=Alu.add)
                nc.vector.tensor_reduce(out=st[:, 1:2], in_=q_cols[:],
                                        axis=mybir.AxisListType.X, op=Alu.add)
                nc.sync.dma_start(out=bnin_d[:, :], in_=st[:])
                # AllGather + local sum is cheaper than AllReduce (1.875x
                # collective-cost factor) for this tiny payload
                nc.gpsimd.collective_compute(
                    "AllGather", Alu.bypass, replica_groups=RG,
                    ins=[bnin_d.ap().opt()], outs=[bnout_d.ap().opt()])
                g8 = spool.tile([D, 2 * NCORES], f32, tag="bng8")
                nc.sync.dma_start(
                    out=g8[:].rearrange("r (c s) -> r c s", s=2),
                    in_=bnout_d[:, :].rearrange("(c r) s -> r c s",
                                                c=NCORES))
                g = spool.tile([D, 2], f32, tag="bng")
                nc.vector.tensor_reduce(
                    out=g[:],
                    in_=g8[:].rearrange("r (c s) -> r s c", s=2),
                    axis=mybir.AxisListType.X, op=Alu.add)
                # mu = (S - corr)/N ; var = (SQ - corrsq)/N - mu^2
                t_a = spool.tile([D, 1], f32, tag="bnta")
                nc.vector.tensor_tensor(out=t_a[:], in0=g[:, 0:1],
                                        in1=bn_s[:, 3:4], op=Alu.subtract)
                mu = spool.tile([D, 1], f32, tag="bnmu")
                nc.vector.tensor_scalar(out=mu[:], in0=t_a[:],
                                        scalar1=1.0 / N_REAL, scalar2=None,
                                        op0=Alu.mult)
                t_b = spool.tile([D, 1], f32, tag="bntb")
                nc.vector.tensor_tensor(out=t_b[:], in0=g[:, 1:2],
                                        in1=bn_s[:, 4:5], op=Alu.subtract)
                msq = spool.tile([D, 1], f32, tag="bnmsq")
                nc.vector.tensor_scalar(out=msq[:], in0=t_b[:],
                                        scalar1=1.0 / N_REAL, scalar2=None,
                                        op0=Alu.mult)
                mu2 = spool.tile([D, 1], f32, tag="bnmu2")
                nc.vector.tensor_tensor(out=mu2[:], in0=mu[:], in1=mu[:],
                                        op=Alu.mult)
                var = spool.tile([D, 1], f32, tag="bnvar")
                nc.vector.tensor_tensor(out=var[:], in0=msq[:], in1=mu2[:],
                                        op=Alu.subtract)
                sd = spool.tile([D, 1], f32, tag="bnsd")
                nc.scalar.activation(out=sd[:], in_=var[:], func=Act.Sqrt,
                                     bias=bn_s[:, 5:6])
                rs = spool.tile([D, 1], f32, tag="bnrs")
                nc.vector.reciprocal(out=rs[:], in_=sd[:])
                scl = spool.tile([D, 1], f32, tag="bnscl")
                nc.vector.tensor_tensor(out=scl[:], in0=bn_s[:, 1:2],
                                        in1=rs[:], op=Alu.mult)
                t_c = spool.tile([D, 1], f32, tag="bntc")
                nc.vector.tensor_tensor(out=t_c[:], in0=mu[:], in1=scl[:],
                                        op=Alu.mult)
                shf = spool.tile([D, 1], f32, tag="bnshf")
                nc.vector.tensor_tensor(out=shf[:], in0=bn_s[:, 2:3],
                                        in1=t_c[:], op=Alu.subtract)
                return scl, shf

            def norm_elu_blockwise(dst_s, src_s, scl, shf, D, do_elu):
                for b in range(0, NB, 2):
                    nb = min(2, NB - b)
                    nsl = slice(b * BLK, (b + nb) * BLK)
                    nw = nb * BLK
                    if not do_elu:
                        nc.vector.tensor_scalar(
                            out=dst_s[:, nsl], in0=src_s[:, nsl],
                            scalar1=scl[:], scalar2=shf[:],
                            op0=Alu.mult, op1=Alu.add)
                        continue
                    hb = spool.tile([D, 2 * BLK], f32, tag="nrmh")
                    nc.vector.tensor_scalar(
                        out=hb[:, 0:nw], in0=src_s[:, nsl], scalar1=scl[:],
                        scalar2=shf[:], op0=Alu.mult, op1=Alu.add)
                    m_t = spool.tile([D, 2 * BLK], f32, tag="nrmm")
                    nc.vector.tensor_scalar(out=m_t[:, 0:nw], in0=hb[:, 0:nw],
                                            scalar1=0.0, scalar2=None,
                                            op0=Alu.min)
                    e_t = spool.tile([D, 2 * BLK], f32, tag="nrme")
                    nc.scalar.activation(out=e_t[:, 0:nw], in_=m_t[:, 0:nw],
                                         func=Act.Exp)
                    nc.vector.tensor_scalar(out=m_t[:, 0:nw], in0=hb[:, 0:nw],
                                            scalar1=0.0, scalar2=None,
                                            op0=Alu.max)
                    nc.vector.tensor_tensor(out=e_t[:, 0:nw],
                                            in0=e_t[:, 0:nw],
                                            in1=m_t[:, 0:nw], op=Alu.add)
                    nc.vector.tensor_scalar(out=dst_s[:, nsl],
                                            in0=e_t[:, 0:nw],
                                            scalar1=-1.0, scalar2=None,
                                            op0=Alu.add)

            # ================= layer 1 =================
            gemm1_replicated()
            edge_layer(H1, F1, fs1p_full, fd1p, fsidx_s, a1_s, r1_s, bn1_s,
                       h1_s, do_elu=True)
            scl1, shf1 = bn_norm(h1_s, bn1_s, bnin[0], bnout[0], HID)
            norm_elu_blockwise(h1_s, h1_s, scl1, shf1, HID, do_elu=True)

            # ================= layer 2 =================
            gemm_layer(lambda t: h1_s[:, t * 128:(t + 1) * 128], HID, w2_s,
                       b2f_s, fd2p, fs2p_loc, fs2p_full)
            edge_layer(H2, F2, fs2p_full, fd2p, fsidx2_s, a2_s, r2_s, bn2_s,
                       h1_s, do_elu=False)  # reuse h1_s as h2 buffer
            scl2, shf2 = bn_norm(h1_s, bn2_s, bnin[1], bnout[1], OUT_DIM)
            norm_elu_blockwise(h1_s, h1_s, scl2, shf2, OUT_DIM, do_elu=False)
            nc.sync.dma_start(out=out_d[:, :], in_=h1_s[0:OUT_DIM, :])

    return nc


_PROGRAM_CACHE = {}


def kernel(**inputs) -> np.ndarray:
    import sys
    for p in ("/opt/trn_rl_repo",):
        if os.path.isdir(p) and p not in sys.path:
            sys.path.insert(0, p)
    from concourse.bass_utils import run_bass_kernel_spmd

    in_maps = make_inputs(inputs)  # sets geometry (TW) from the graph
    key = ("nc", TW)
    if key not in _PROGRAM_CACHE:
        nc = build_program()
        nc.finalize()
        _PROGRAM_CACHE[key] = nc
    nc = _PROGRAM_CACHE[key]
    res = run_bass_kernel_spmd(nc, in_maps, core_ids=list(range(NCORES)))
    vcol2 = np.array([(c % F2) * H2 + c // F2 for c in range(OUT_DIM)])
    outs = [res.results[c]["outT"].T[:, vcol2]  # un-permute v-order rows
            for c in range(NCORES)]
    return np.ascontiguousarray(np.concatenate(outs, 0)[:N_REAL]).astype(
        np.float32)


if __name__ == "__main__":
    import jax
    with jax.default_device(jax.devices("cpu")[0]):
        import reference
        inputs = {k: np.asarray(v) for k, v in reference.setup_inputs().items()}
        expected = np.asarray(reference.reference(**inputs))
    actual = kernel(**inputs)
    rel = np.linalg.norm(actual - expected) / np.linalg.norm(expected)
    print("Relative error:", rel)



# revision 18
# speedup vs baseline: 1.0540x; 1.0540x over previous
"""Trainium2 Bass kernel for a 2-layer GATv2 encoder (nn_GATv2Encoder).

Strategy (8 NeuronCores, SPMD):
- Host sorts edges by dst; each core owns a contiguous 6400-dst-node range,
  so segment softmax/scatter are core-local. Within a core, dst nodes form
  256-node blocks; each block's edges are split by src parity into two
  sections, and each section's edges are grouped into four 64-node dst
  windows with a fixed (data-derived) tile count TW per window.
- Node-feature tables are bf16, node-pair-packed ([25600, 128]: row r =
  [f(2r)|f(2r+1)]), so int16 gather indices cover all 51200 padded nodes
  (idx = node>>1, parity selects the 64-column half). fd tables are local
  bf16 [6400, 128] rows.
- Layer-1 projections: every core computes the FULL graph GEMM from a
  per-core node-rotated bf16 x input (own nodes first) -- no AllGather at
  all; bias is added with a 1-row ones matmul into the same psum group.
  Layer-2 projections are data-parallel with a bf16 packed-fs AllGather.
- Edge phase per block: one merged dma_gather each for fs/fd (both
  sections), bf16 z-path (z = fs+fd, |z| on ACT, attn mult, head-reduce)
  running mostly in DVE 2x mode, exp on ACT. Scatter-add is a banded
  one-hot matmul: the one-hot is window-relative ([128, 64, T]) and built
  n-major in one bf16 2x DVE op; per-window psum accumulation groups open/
  close contiguously in a [72, 256] bank (num rows 0:64, den rows 64:72).
- Node features use an f-major ("v-order") column layout so the vals
  multiply broadcasts along the last (head) dim and stays in DVE 2x mode;
  host permutes all weights/consts and un-permutes the final output. The
  per-head logit reduce is an in-place pairwise bf16 tree of 2x adds.
- Softmax normalization folds into num*(1/den); BatchNorm stats use a tiny
  AllGather + local sum (cheaper than AllReduce), with padded-row
  corrections precomputed on host.
"""

import os
import numpy as np

# ---------------------------------------------------------------- constants
N_REAL = 50000
E_REAL = 800000
IN_DIM, HID, OUT_DIM = 128, 64, 64
H1, F1 = 8, 8
H2, F2 = 4, 16
SLOPE = 0.2
EPS = 1e-5
DEN_EPS = 1e-30

NCORES = 8
BLK = 256                      # nodes per block (one-hot / psum free width)
NB = 25                        # blocks per core
CORE_NODES = BLK * NB          # 6400
NPAD = NCORES * CORE_NODES     # 51200
HALF = NPAD // 2               # 25600 (int16-safe gather split)
TILE = 128                     # edges per matmul tile
WIN = 64                       # scatter one-hot band window (nodes)
NW = BLK // WIN                # windows per block (4)
# TW (tiles per window), SEC_T, CAP etc. are data-dependent; host_prep sets
# them via _set_geometry before build_program runs.
TW = None
SEC_T = None
CAP = None
T_BLK = None
SEC_COLS = None
BLK_COLS = None


def _set_geometry(tw):
    global TW, SEC_T, CAP, T_BLK, SEC_COLS, BLK_COLS
    TW = tw
    SEC_T = NW * tw
    CAP = SEC_T * TILE
    T_BLK = 2 * SEC_T
    SEC_COLS = CAP // 16
    BLK_COLS = 2 * SEC_COLS


# ---------------------------------------------------------------- host prep
def _wrap16(idx_list, cols):
    """int16 indices in dma_gather layout: [128, cols]; idx j at
    (partition j%16, col j//16), replicated across the 8 Q7 core groups."""
    flat = np.zeros(16 * cols, np.int16)
    flat[: len(idx_list)] = np.asarray(idx_list, np.int16)
    a = np.ascontiguousarray(flat.reshape(cols, 16).T)  # idx j -> (j%16, j//16)
    return np.tile(a, (8, 1))


def host_prep(src, dst):
    src = np.asarray(src).astype(np.int64)
    dst = np.asarray(dst).astype(np.int64)
    order = np.argsort(dst, kind="stable")
    s_src, s_dst = src[order], dst[order]
    counts = np.bincount(s_dst, minlength=NPAD)
    assert counts[:N_REAL].min() > 0, "zero in-degree node"
    cum = np.concatenate([[0], np.cumsum(counts)])

    # pass 1: per (core, block, parity, window) edge lists + max tile count
    secs = {}
    tw = 1
    for c in range(NCORES):
        base = c * CORE_NODES
        for b in range(NB):
            nb0 = base + b * BLK
            e0, e1 = cum[nb0], cum[min(nb0 + BLK, NPAD)]
            bs, bd = s_src[e0:e1], s_dst[e0:e1]
            rp = (bs - base) % NPAD          # rotated src position
            for sec in range(2):
                sel = (rp % 2) == sec
                ss, sg, dd = rp[sel], bs[sel], bd[sel] - nb0
                for w in range(NW):
                    m = (dd // WIN) == w
                    secs[(c, b, sec, w)] = (ss[m], sg[m], dd[m])
                    tw = max(tw, (len(ss[m]) + TILE - 1) // TILE)
    _set_geometry(tw)

    cores = []
    for c in range(NCORES):
        base = c * CORE_NODES
        fs_cols, fs2_cols, fd_cols = [], [], []
        dloc = np.full((128, NB * T_BLK), -1.0, np.float32)
        for b in range(NB):
            nb0 = base + b * BLK
            for sec in range(2):
                fsl = np.zeros(CAP, np.int64)
                fs2l = np.zeros(CAP, np.int64)
                fdl = np.zeros(CAP, np.int64)
                dl = np.full(CAP, -1.0, np.float32)
                for w in range(NW):
                    ss, sg, dd = secs[(c, b, sec, w)]
                    o = w * TW * TILE
                    fsl[o:o + len(ss)] = ss >> 1
                    fs2l[o:o + len(sg)] = sg >> 1
                    fdl[o:o + len(dd)] = dd + nb0 - base
                    dl[o:o + len(dd)] = dd - w * WIN
                fs_cols.append(_wrap16(fsl, SEC_COLS))
                fs2_cols.append(_wrap16(fs2l, SEC_COLS))
                fd_cols.append(_wrap16(fdl, SEC_COLS))
                dcol = (b * 2 + sec) * SEC_T
                dloc[:, dcol:dcol + SEC_T] = dl.reshape(SEC_T, 128).T
        cores.append(dict(
            fsidx=np.concatenate(fs_cols, 1),
            fsidx2=np.concatenate(fs2_cols, 1),
            fdidx=np.concatenate(fd_cols, 1),
            dloc=dloc,
        ))
    return cores


def _elu_np(x):
    return np.where(x > 0, x, np.exp(np.minimum(x, 0)) - 1).astype(np.float32)


def make_inputs(inputs):
    """Build the 8 per-core input maps for the bass program."""
    f32 = np.float32
    x = np.asarray(inputs["x"], f32)
    cores = host_prep(inputs["src"], inputs["dst"])

    xp = np.zeros((NPAD, IN_DIM), f32)
    xp[:N_REAL] = x

    w1 = np.concatenate([np.asarray(inputs["Wsrc1"], f32),
                         np.asarray(inputs["Wdst1"], f32)], 1)   # [128,128]
    w2 = np.concatenate([np.asarray(inputs["Wsrc2"], f32),
                         np.asarray(inputs["Wdst2"], f32)], 1)   # [64,128]
    b1 = np.concatenate([np.asarray(inputs["bsrc1"], f32),
                         np.asarray(inputs["bdst1"], f32)])      # [128]
    b2 = np.concatenate([np.asarray(inputs["bsrc2"], f32),
                         np.asarray(inputs["bdst2"], f32)])
    b1full = np.tile(b1[None, :], (128, 1)).astype(f32)
    b2full = np.tile(b2[None, :], (128, 1)).astype(f32)

    # lrelu(z) = (1+s)/2*z + (1-s)/2*|z|; the (1+s)/2 factor is folded into
    # the attention constants, the |z| path uses ACT Abs with scale (1-s)/(1+s)
    import ml_dtypes
    bf16 = ml_dtypes.bfloat16
    lr_a = (1.0 + SLOPE) / 2.0
    attn1f = np.tile(np.asarray(inputs["attn1"], f32).reshape(1, -1) * lr_a,
                     (128, 1)).astype(bf16)
    attn2f = np.tile(np.asarray(inputs["attn2"], f32).reshape(1, -1) * lr_a,
                     (128, 1)).astype(bf16)
    # n-major iota over the band window: iota[p, n*T_BLK + t] = n
    iota = np.tile(np.repeat(np.arange(WIN, dtype=f32), T_BLK)[None, :],
                   (128, 1)).astype(bf16)
    w1b = np.concatenate([np.asarray(inputs["Wsrc1"], f32),
                          np.asarray(inputs["Wdst1"], f32)], 1).astype(bf16)
    ones1 = np.ones((1, 128), bf16)
    b1row = b1[None, :].astype(bf16)

    # v-order (f-major) feature layout: v-col v holds std feature
    # cperm[v] = (v%H)*F + v//H; enables DVE 2x on the vals multiply.
    cperm1 = np.array([(v % H1) * F1 + v // H1 for v in range(HID)])
    cperm2 = np.array([(v % H2) * F2 + v // H2 for v in range(OUT_DIM)])
    pp1 = np.r_[cperm1, HID + cperm1]
    pp2 = np.r_[cperm2, OUT_DIM + cperm2]
    w1b = w1b[:, pp1]
    b1row = b1row[:, pp1]
    w2 = np.ascontiguousarray(w2[cperm1][:, pp2])
    b2full = np.ascontiguousarray(b2full[:, pp2])
    attn1f = np.ascontiguousarray(attn1f[:, cperm1])
    attn2f = np.ascontiguousarray(attn2f[:, cperm2])
    r1 = np.zeros((H1, HID), f32)
    r1[np.arange(HID) % H1, np.arange(HID)] = 1.0
    r2 = np.zeros((H2, OUT_DIM), f32)
    r2[np.arange(OUT_DIM) % H2, np.arange(OUT_DIM)] = 1.0

    npad_rows = NPAD - N_REAL
    bias1 = np.asarray(inputs["bias1"], f32)
    bias2 = np.asarray(inputs["bias2"], f32)
    cpad1 = _elu_np(bias1)
    bn1 = np.zeros((HID, 8), f32)
    bn1[:, 0] = bias1
    bn1[:, 1] = np.asarray(inputs["gamma1"], f32)
    bn1[:, 2] = np.asarray(inputs["beta1"], f32)
    bn1[:, 3] = npad_rows * cpad1
    bn1[:, 4] = npad_rows * cpad1 ** 2
    bn1[:, 5] = EPS
    bn2 = np.zeros((OUT_DIM, 8), f32)
    bn2[:, 0] = bias2
    bn2[:, 1] = np.asarray(inputs["gamma2"], f32)
    bn2[:, 2] = np.asarray(inputs["beta2"], f32)
    bn2[:, 3] = npad_rows * bias2
    bn2[:, 4] = npad_rows * bias2 ** 2
    bn2[:, 5] = EPS
    bn1 = np.ascontiguousarray(bn1[cperm1])
    bn2 = np.ascontiguousarray(bn2[cperm2])

    in_maps = []
    for c in range(NCORES):
        base = c * CORE_NODES
        xrot = np.roll(xp, -base, axis=0)  # my nodes first (rotated order)
        in_maps.append({
            "xT": np.ascontiguousarray(xrot.T).astype(bf16),
            "w1b": w1b, "ones1": ones1, "b1row": b1row,
            "w2": w2, "b2full": b2full,
            "attn1f": attn1f, "attn2f": attn2f,
            "iota": iota, "r1": r1, "r2": r2, "bn1": bn1, "bn2": bn2,
            "fsidx": cores[c]["fsidx"], "fsidx2": cores[c]["fsidx2"],
            "fdidx": cores[c]["fdidx"],
            "dloc": cores[c]["dloc"].astype(bf16),
        })
    return in_maps


# ---------------------------------------------------------------- bass program
def build_program():
    import concourse.bacc as bacc
    import concourse.tile as tile
    from concourse import mybir

    f32 = mybir.dt.float32
    bf16 = mybir.dt.bfloat16
    i16 = mybir.dt.int16
    Alu = mybir.AluOpType
    Act = mybir.ActivationFunctionType

    nc = bacc.Bacc(None, target_bir_lowering=False, num_devices=NCORES)
    RG = [list(range(NCORES))]

    # ---- I/O ----
    xT_d = nc.dram_tensor("xT", [IN_DIM, NPAD], bf16, kind="ExternalInput")
    w1b_d = nc.dram_tensor("w1b", [IN_DIM, 128], bf16, kind="ExternalInput")
    ones1_d = nc.dram_tensor("ones1", [1, 128], bf16, kind="ExternalInput")
    b1row_d = nc.dram_tensor("b1row", [1, 128], bf16, kind="ExternalInput")
    w2_d = nc.dram_tensor("w2", [HID, 128], f32, kind="ExternalInput")
    b2f_d = nc.dram_tensor("b2full", [128, 128], f32, kind="ExternalInput")
    a1_d = nc.dram_tensor("attn1f", [128, HID], bf16, kind="ExternalInput")
    a2_d = nc.dram_tensor("attn2f", [128, OUT_DIM], bf16, kind="ExternalInput")
    iota_d = nc.dram_tensor("iota", [128, WIN * T_BLK], bf16,
                            kind="ExternalInput")
    r1_d = nc.dram_tensor("r1", [H1, HID], f32, kind="ExternalInput")
    r2_d = nc.dram_tensor("r2", [H2, OUT_DIM], f32, kind="ExternalInput")
    bn1_d = nc.dram_tensor("bn1", [HID, 8], f32, kind="ExternalInput")
    bn2_d = nc.dram_tensor("bn2", [OUT_DIM, 8], f32, kind="ExternalInput")
    fsidx_d = nc.dram_tensor("fsidx", [128, NB * BLK_COLS], i16, kind="ExternalInput")
    fsidx2_d = nc.dram_tensor("fsidx2", [128, NB * BLK_COLS], i16, kind="ExternalInput")
    fdidx_d = nc.dram_tensor("fdidx", [128, NB * BLK_COLS], i16, kind="ExternalInput")
    dloc_d = nc.dram_tensor("dloc", [128, NB * T_BLK], bf16,
                            kind="ExternalInput")
    out_d = nc.dram_tensor("outT", [OUT_DIM, CORE_NODES], f32, kind="ExternalOutput")

    # ---- internal DRAM ----
    # bf16 node-pair-packed fs tables: row r = [fs(2r) | fs(2r+1)]
    # (layer 1 computed fully on-core in rotated order; layer 2 AllGathered)
    fs1p_full = nc.dram_tensor("fs1p_full", [NPAD // 2, 128], bf16)
    fs2p_loc = nc.dram_tensor("fs2p_loc", [CORE_NODES // 2, 128], bf16)
    fs2p_full = nc.dram_tensor("fs2p_full", [NPAD // 2, 128], bf16,
                               addr_space="Shared")
    # bf16 fd tables, 256B rows [fd(n) | junk]
    fd1p = nc.dram_tensor("fd1p", [CORE_NODES, 128], bf16)
    fd2p = nc.dram_tensor("fd2p", [CORE_NODES, 128], bf16)
    bnin = [nc.dram_tensor(f"bnin{i}", [64, 2], f32) for i in (1, 2)]
    bnout = [nc.dram_tensor(f"bnout{i}", [64 * NCORES, 2], f32,
                            addr_space="Shared") for i in (1, 2)]

    with tile.TileContext(nc) as tc:
        with (
            tc.tile_pool(name="const", bufs=1) as cpool,
            tc.tile_pool(name="gath", bufs=2) as gpool,
            tc.tile_pool(name="work", bufs=2) as wpool,
            tc.tile_pool(name="small", bufs=2) as spool,
            tc.tile_pool(name="node", bufs=1) as npool,
            tc.tile_pool(name="psA", bufs=2, space="PSUM") as psA,
            tc.tile_pool(name="psB", bufs=2, space="PSUM") as psB,
            tc.tile_pool(name="psG", bufs=2, space="PSUM") as psG,
        ):
            # ---- load constants ----
            def load(dram, shape, dtype=f32, pool=cpool):
                t = pool.tile(shape, dtype, tag=f"c_{dram.name}")
                nc.sync.dma_start(out=t[:], in_=dram[:, :])
                return t

            w1b_s = load(w1b_d, [IN_DIM, 128], bf16)
            ones1_s = load(ones1_d, [1, 128], bf16)
            b1row_s = load(b1row_d, [1, 128], bf16)
            w2_s = load(w2_d, [HID, 128])
            b2f_s = load(b2f_d, [128, 128])
            a1_s = load(a1_d, [128, HID], bf16)
            a2_s = load(a2_d, [128, OUT_DIM], bf16)
            iota_s = load(iota_d, [128, WIN * T_BLK], bf16)
            r1_s = load(r1_d, [H1, HID])
            r2_s = load(r2_d, [H2, OUT_DIM])
            bn1_s = load(bn1_d, [HID, 8])
            bn2_s = load(bn2_d, [OUT_DIM, 8])
            fsidx_s = load(fsidx_d, [128, NB * BLK_COLS], i16)
            fsidx2_s = load(fsidx2_d, [128, NB * BLK_COLS], i16)
            fdidx_s = load(fdidx_d, [128, NB * BLK_COLS], i16)
            dloc_s = load(dloc_d, [128, NB * T_BLK], bf16)

            h1_s = npool.tile([HID, CORE_NODES], f32, tag="h1")

            NT = CORE_NODES // 128  # node tiles per core for GEMMs

            def gemm_layer(get_lhs, K, w_s, bfull_s, fdp, pack_loc,
                           pack_full):
                # batched 4 node-tiles per psum bank
                t0 = 0
                while t0 < NT:
                    nt = min(4, NT - t0)
                    ps = psG.tile([128, 512], f32)
                    for k in range(nt):
                        sl = slice(k * 128, (k + 1) * 128)
                        nc.tensor.matmul(
                            out=ps[:, sl], lhsT=get_lhs(t0 + k),
                            rhs=w_s[:K, :], start=True, stop=True)
                    ps4 = ps[:].rearrange("p (t c) -> p t c", c=128)
                    gs = spool.tile([128, 512], f32, tag="gemm")
                    gs4 = gs[:].rearrange("p (t c) -> p t c", c=128)
                    nc.vector.tensor_tensor(
                        out=gs4[:, 0:nt, :], in0=ps4[:, 0:nt, :],
                        in1=bfull_s[:, None, :]
                            .to_broadcast([128, nt, 128]),
                        op=Alu.add)
                    # pack fs half to bf16 pair rows: row r=[fs(2r)|fs(2r+1)]
                    gp = spool.tile([128, 4 * 64], bf16, tag="gpack")
                    gp3 = gp[:].rearrange("p (t e) -> p t e", e=64)
                    nc.vector.tensor_copy(out=gp3[:, 0:nt, :],
                                          in_=gs4[:, 0:nt, 0:64])
                    nc.sync.dma_start(
                        out=pack_loc[t0 * 64:(t0 + nt) * 64, :]
                            .rearrange("(t r) (q e) -> (r q) t e", t=nt, q=2),
                        in_=gp3[:, 0:nt, :])
                    gpd = spool.tile([128, 4 * 64], bf16, tag="gpackd")
                    gpd3 = gpd[:].rearrange("p (t e) -> p t e", e=64)
                    nc.vector.tensor_copy(out=gpd3[:, 0:nt, :],
                                          in_=gs4[:, 0:nt, 64:128])
                    nc.sync.dma_start(
                        out=fdp[t0 * 128:(t0 + nt) * 128, 0:64]
                            .rearrange("(t r) e -> r t e", t=nt),
                        in_=gpd3[:, 0:nt, :])
                    t0 += nt
                nc.gpsimd.collective_compute(
                    "AllGather", Alu.bypass, replica_groups=RG,
                    ins=[pack_loc.ap().opt()], outs=[pack_full.ap().opt()])

            def gemm1_replicated():
                # full-graph GEMM on every core (rotated node order): no
                # collective needed; bias added via a 1-row ones matmul.
                # Batched 4 node-tiles per psum bank to amortize DMA/DVE.
                for t4 in range(NPAD // 512):
                    xg = spool.tile([IN_DIM, 512], bf16, tag="xg")
                    nc.sync.dma_start(
                        out=xg[:], in_=xT_d[:, t4 * 512:(t4 + 1) * 512])
                    ps = psG.tile([128, 512], f32)
                    for k in range(4):
                        sl = slice(k * 128, (k + 1) * 128)
                        nc.tensor.matmul(out=ps[:, sl], lhsT=xg[:, sl],
                                         rhs=w1b_s[:], start=True, stop=False)
                        nc.tensor.matmul(out=ps[:, sl], lhsT=ones1_s[:],
                                         rhs=b1row_s[:], start=False,
                                         stop=True)
                    ps4 = ps[:].rearrange("p (t c) -> p t c", c=128)
                    gp = spool.tile([128, 4 * 64], bf16, tag="gpack")
                    nc.vector.tensor_copy(
                        out=gp[:].rearrange("p (t e) -> p t e", e=64),
                        in_=ps4[:, :, 0:64])
                    nc.sync.dma_start(
                        out=fs1p_full[t4 * 256:(t4 + 1) * 256, :]
                            .rearrange("(t r) (q e) -> (r q) t e", t=4, q=2),
                        in_=gp[:].rearrange("p (t e) -> p t e", e=64))
                    nloc = min(max(CORE_NODES - t4 * 512, 0), 512) // 128
                    if nloc > 0:  # local-node fd table rows (may be partial)
                        gpd = spool.tile([128, 4 * 64], bf16, tag="gpackd")
                        nc.vector.tensor_copy(
                            out=gpd[:].rearrange("p (t e) -> p t e", e=64),
                            in_=ps4[:, :, 64:128])
                        nc.sync.dma_start(
                            out=fd1p[t4 * 512:t4 * 512 + nloc * 128, 0:64]
                                .rearrange("(t r) e -> r t e", t=nloc),
                            in_=gpd[:].rearrange("p (t e) -> p t e",
                                                 e=64)[:, 0:nloc, :])

            def edge_layer(Hh, Ff, pack_full, fdp, fsix_s, attn_s, rX_s,
                           bn_s, hout_s, do_elu):
                HF = Hh * Ff
                VW = Hh + HF  # vals width per tile
                for b in range(NB):
                    cA = b * BLK_COLS
                    fs_e = gpool.tile([128, T_BLK * 128], bf16, tag="fs")
                    fd_e = gpool.tile([128, T_BLK * 128], bf16, tag="fd")
                    nc.gpsimd.dma_gather(
                        out_ap=fs_e[:].rearrange("p (t e) -> p t e", e=128),
                        in_ap=pack_full[:, :],
                        idxs_ap=fsix_s[:, cA:cA + 2 * SEC_COLS],
                        num_idxs=2 * CAP, num_idxs_reg=2 * CAP,
                        elem_size=128, elem_step=128, single_packet=False)
                    nc.gpsimd.dma_gather(
                        out_ap=fd_e[:].rearrange("p (t e) -> p t e", e=128),
                        in_ap=fdp[:, :],
                        idxs_ap=fdidx_s[:, cA:cA + 2 * SEC_COLS],
                        num_idxs=2 * CAP, num_idxs_reg=2 * CAP,
                        elem_size=128, elem_step=128, single_packet=False)
                    fs3 = fs_e[:].rearrange("p (t e) -> p t e", e=128)
                    fd3 = fd_e[:].rearrange("p (t e) -> p t e", e=128)

                    # banded one-hot (bf16, n-major): window-relative
                    dcol = b * T_BLK
                    O_t = wpool.tile([128, WIN * T_BLK], bf16, tag="O")
                    O3 = O_t[:].rearrange("p (n t) -> p n t", t=T_BLK)
                    nc.vector.tensor_tensor(
                        out=O3,
                        in0=dloc_s[:, None, dcol:dcol + T_BLK]
                            .to_broadcast([128, WIN, T_BLK]),
                        in1=iota_s[:].rearrange("p (n t) -> p n t", t=T_BLK),
                        op=Alu.is_equal)

                    # z = fs + fd ; lrelu(z)*attn = (z + c|z|) * attn06
                    z_t = wpool.tile([128, T_BLK * 64], bf16, tag="z")
                    wz_t = wpool.tile([128, T_BLK * 64], bf16, tag="wz")
                    z3 = z_t[:].rearrange("p (t e) -> p t e", e=64)
                    for sec in range(2):
                        ts = slice(sec * SEC_T, (sec + 1) * SEC_T)
                        nc.vector.tensor_tensor(
                            out=z3[:, ts, :],
                            in0=fs3[:, ts, 64 * sec:64 * sec + 64],
                            in1=fd3[:, ts, 0:64], op=Alu.add)
                    nc.scalar.activation(
                        out=wz_t[:], in_=z_t[:], func=Act.Abs,
                        scale=(1.0 - SLOPE) / (1.0 + SLOPE))
                    nc.vector.tensor_tensor(
                        out=wz_t[:], in0=z_t[:], in1=wz_t[:], op=Alu.add)
                    nc.vector.tensor_tensor(
                        out=z3,
                        in0=wz_t[:].rearrange("p (t e) -> p t e", e=64),
                        in1=attn_s[:, None, :]
                            .to_broadcast([128, T_BLK, 64]),
                        op=Alu.mult)
                    # l = sum_f wz ; p = exp(l) into vals
                    # l[e,h] = sum_f zl[e,(f,h)] via in-place pairwise tree
                    # (bf16 2x adds beat the 1x tensor_reduce)
                    zv = z_t[:].rearrange("p (t v) -> p t v", v=64)
                    fw = Ff
                    while fw > 1:
                        half = (fw // 2) * Hh
                        nc.vector.tensor_tensor(
                            out=zv[:, :, 0:half], in0=zv[:, :, 0:half],
                            in1=zv[:, :, half:2 * half], op=Alu.add)
                        fw //= 2
                    vals = wpool.tile([128, T_BLK * VW], bf16, tag="vals")
                    vals3 = vals[:].rearrange("p (t v) -> p t v", v=VW)
                    nc.scalar.activation(
                        out=vals3[:, :, HF:VW],
                        in_=zv[:, :, 0:Hh],
                        func=Act.Exp)
                    for sec in range(2):
                        ts = slice(sec * SEC_T, (sec + 1) * SEC_T)
                        nc.vector.tensor_tensor(
                            out=vals3[:, ts, 0:HF].rearrange(
                                "p t (f h) -> p t f h", h=Hh),
                            in0=fs3[:, ts, 64 * sec:64 * sec + 64].rearrange(
                                "p t (f h) -> p t f h", h=Hh),
                            in1=vals3[:, ts, HF:VW][:, :, None, :]
                                .to_broadcast([128, SEC_T, Ff, Hh]),
                            op=Alu.mult)

                    # windowed scatter, window-major (contiguous psum groups)
                    ps_s = psA.tile([VW, BLK], f32, tag="scat")
                    for w in range(NW):
                        for sec in range(2):
                            for tt in range(TW):
                                t = sec * SEC_T + w * TW + tt
                                nc.tensor.matmul(
                                    out=ps_s[:, w * WIN:(w + 1) * WIN],
                                    lhsT=vals[:, t * VW:(t + 1) * VW],
                                    rhs=O3[:, :, t],
                                    start=(sec == 0 and tt == 0),
                                    stop=(sec == 1 and tt == TW - 1))

                    # normalize: out = num * (1/den) + bias
                    den = spool.tile([Hh, BLK], f32, tag="den")
                    nc.vector.tensor_scalar(
                        out=den[:], in0=ps_s[HF:VW, :], scalar1=DEN_EPS,
                        scalar2=None, op0=Alu.add)
                    rcp = spool.tile([Hh, BLK], f32, tag="rcp")
                    nc.vector.reciprocal(out=rcp[:], in_=den[:])
                    ps_r = psB.tile([HF, BLK], f32, tag="rrep")
                    nc.tensor.matmul(out=ps_r[:], lhsT=rX_s[:], rhs=rcp[:],
                                     start=True, stop=True)
                    rr = spool.tile([HF, BLK], f32, tag="rr")
                    nc.vector.tensor_copy(out=rr[:], in_=ps_r[:])
                    o1 = spool.tile([HF, BLK], f32, tag="o1")
                    nc.vector.tensor_tensor(
                        out=o1[:], in0=ps_s[0:HF, :], in1=rr[:], op=Alu.mult)
                    nsl = slice(b * BLK, (b + 1) * BLK)
                    if do_elu:
                        ob = spool.tile([HF, BLK], f32, tag="ob")
                        nc.vector.tensor_scalar(
                            out=ob[:], in0=o1[:], scalar1=bn_s[:, 0:1],
                            scalar2=None, op0=Alu.add)
                        m_t = spool.tile([HF, BLK], f32, tag="elum")
                        nc.vector.tensor_scalar(
                            out=m_t[:], in0=ob[:], scalar1=0.0,
                            scalar2=None, op0=Alu.min)
                        e_t = spool.tile([HF, BLK], f32, tag="elue")
                        nc.scalar.activation(out=e_t[:], in_=m_t[:],
                                             func=Act.Exp)
                        nc.vector.tensor_scalar(
                            out=m_t[:], in0=ob[:], scalar1=0.0,
                            scalar2=None, op0=Alu.max)
                        t_t = spool.tile([HF, BLK], f32, tag="elut")
                        nc.vector.tensor_tensor(
                            out=t_t[:], in0=e_t[:], in1=m_t[:], op=Alu.add)
                        nc.vector.tensor_scalar(
                            out=hout_s[:, nsl], in0=t_t[:], scalar1=-1.0,
                            scalar2=None, op0=Alu.add)
                    else:
                        nc.vector.tensor_scalar(
                            out=hout_s[:, nsl], in0=o1[:], scalar1=bn_s[:, 0:1],
                            scalar2=None, op0=Alu.add)

            def bn_norm(hin_s, bn_s, bnin_d, bnout_d, D):
                """BN stats (blockwise) + AllReduce; returns (scale, shift)."""
                NBB = (NB + 1) // 2
                s_cols = spool.tile([D, NBB], f32, tag="bnscols")
                q_cols = spool.tile([D, NBB], f32, tag="bnqcols")
                for i, b in enumerate(range(0, NB, 2)):
                    nb = min(2, NB - b)
                    nsl = slice(b * BLK, (b + nb) * BLK)
                    nc.vector.tensor_reduce(
                        out=s_cols[:, i:i + 1], in_=hin_s[:, nsl],
                        axis=mybir.AxisListType.X, op=Alu.add)
                    scr = spool.tile([D, 2 * BLK], f32, tag="nrmh")
                    nc.scalar.activation(
                        out=scr[:, 0:nb * BLK], in_=hin_s[:, nsl],
                        func=Act.Square, accum_out=q_cols[:, i:i + 1])
                st = spool.tile([D, 2], f32, tag="bnst")
                nc.vector.tensor_reduce(out=st[:, 0:1], in_=s_cols[:],
                                        axis=mybir.AxisListType.X, op=Alu.add)
                nc.vector.tensor_reduce(out=st[:, 1:2], in_=q_cols[:],
                                        axis=mybir.AxisListType.X, op=Alu.add)
                nc.sync.dma_start(out=bnin_d[:, :], in_=st[:])
                # AllGather + local sum is cheaper than AllReduce (1.875x
                # collective-cost factor) for this tiny payload
                nc.gpsimd.collective_compute(
                    "AllGather", Alu.bypass, replica_groups=RG,
                    ins=[bnin_d.ap().opt()], outs=[bnout_d.ap().opt()])
                g8 = spool.tile([D, 2 * NCORES], f32, tag="bng8")
                nc.sync.dma_start(
                    out=g8[:].rearrange("r (c s) -> r c s", s=2),
                    in_=bnout_d[:, :].rearrange("(c r) s -> r c s",
                                                c=NCORES))
                g = spool.tile([D, 2], f32, tag="bng")
                nc.vector.tensor_reduce(
                    out=g[:],
                    in_=g8[:].rearrange("r (c s) -> r s c", s=2),
                    axis=mybir.AxisListType.X, op=Alu.add)
                # mu = (S - corr)/N ; var = (SQ - corrsq)/N - mu^2
                t_a = spool.tile([D, 1], f32, tag="bnta")
                nc.vector.tensor_tensor(out=t_a[:], in0=g[:, 0:1],
                                        in1=bn_s[:, 3:4], op=Alu.subtract)
                mu = spool.tile([D, 1], f32, tag="bnmu")
                nc.vector.tensor_scalar(out=mu[:], in0=t_a[:],
                                        scalar1=1.0 / N_REAL, scalar2=None,
                                        op0=Alu.mult)
                t_b = spool.tile([D, 1], f32, tag="bntb")
                nc.vector.tensor_tensor(out=t_b[:], in0=g[:, 1:2],
                                        in1=bn_s[:, 4:5], op=Alu.subtract)
                msq = spool.tile([D, 1], f32, tag="bnmsq")
                nc.vector.tensor_scalar(out=msq[:], in0=t_b[:],
                                        scalar1=1.0 / N_REAL, scalar2=None,
                                        op0=Alu.mult)
                mu2 = spool.tile([D, 1], f32, tag="bnmu2")
                nc.vector.tensor_tensor(out=mu2[:], in0=mu[:], in1=mu[:],
                                        op=Alu.mult)
                var = spool.tile([D, 1], f32, tag="bnvar")
                nc.vector.tensor_tensor(out=var[:], in0=msq[:], in1=mu2[:],
                                        op=Alu.subtract)
                sd = spool.tile([D, 1], f32, tag="bnsd")
                nc.scalar.activation(out=sd[:], in_=var[:], func=Act.Sqrt,
                                     bias=bn_s[:, 5:6])
                rs = spool.tile([D, 1], f32, tag="bnrs")
                nc.vector.reciprocal(out=rs[:], in_=sd[:])
                scl = spool.tile([D, 1], f32, tag="bnscl")
                nc.vector.tensor_tensor(out=scl[:], in0=bn_s[:, 1:2],
                                        in1=rs[:], op=Alu.mult)
                t_c = spool.tile([D, 1], f32, tag="bntc")
                nc.vector.tensor_tensor(out=t_c[:], in0=mu[:], in1=scl[:],
                                        op=Alu.mult)
                shf = spool.tile([D, 1], f32, tag="bnshf")
                nc.vector.tensor_tensor(out=shf[:], in0=bn_s[:, 2:3],
                                        in1=t_c[:], op=Alu.subtract)
                return scl, shf

            def norm_elu_blockwise(dst_s, src_s, scl, shf, D, do_elu):
                for b in range(0, NB, 2):
                    nb = min(2, NB - b)
                    nsl = slice(b * BLK, (b + nb) * BLK)
                    nw = nb * BLK
                    if not do_elu:
                        nc.scalar.activation(
                            out=dst_s[:, nsl], in_=src_s[:, nsl],
                            func=Act.Identity, scale=scl[:], bias=shf[:])
                        continue
                    hb = spool.tile([D, 2 * BLK], f32, tag="nrmh")
                    nc.scalar.activation(
                        out=hb[:, 0:nw], in_=src_s[:, nsl],
                        func=Act.Identity, scale=scl[:], bias=shf[:])
                    m_t = spool.tile([D, 2 * BLK], f32, tag="nrmm")
                    nc.vector.tensor_scalar(out=m_t[:, 0:nw], in0=hb[:, 0:nw],
                                            scalar1=0.0, scalar2=None,
                                            op0=Alu.min)
                    e_t = spool.tile([D, 2 * BLK], f32, tag="nrme")
                    nc.scalar.activation(out=e_t[:, 0:nw], in_=m_t[:, 0:nw],
                                         func=Act.Exp)
                    nc.vector.tensor_scalar(out=m_t[:, 0:nw], in0=hb[:, 0:nw],
                                            scalar1=0.0, scalar2=None,
                                            op0=Alu.max)
                    nc.vector.tensor_tensor(out=e_t[:, 0:nw],
                                            in0=e_t[:, 0:nw],
                                            in1=m_t[:, 0:nw], op=Alu.add)
                    nc.vector.tensor_scalar(out=dst_s[:, nsl],
                                            in0=e_t[:, 0:nw],
                                            scalar1=-1.0, scalar2=None,
                                            op0=Alu.add)

            # ================= layer 1 =================
            gemm1_replicated()
            edge_layer(H1, F1, fs1p_full, fd1p, fsidx_s, a1_s, r1_s, bn1_s,
                       h1_s, do_elu=True)
            scl1, shf1 = bn_norm(h1_s, bn1_s, bnin[0], bnout[0], HID)
            norm_elu_blockwise(h1_s, h1_s, scl1, shf1, HID, do_elu=True)

            # ================= layer 2 =================
            gemm_layer(lambda t: h1_s[:, t * 128:(t + 1) * 128], HID, w2_s,
                       b2f_s, fd2p, fs2p_loc, fs2p_full)
            edge_layer(H2, F2, fs2p_full, fd2p, fsidx2_s, a2_s, r2_s, bn2_s,
                       h1_s, do_elu=False)  # reuse h1_s as h2 buffer
            scl2, shf2 = bn_norm(h1_s, bn2_s, bnin[1], bnout[1], OUT_DIM)
            norm_elu_blockwise(h1_s, h1_s, scl2, shf2, OUT_DIM, do_elu=False)
            for b in range(0, NB, 2):  # chunked: overlaps the norm slabs
                nsl = slice(b * BLK, min((b + 2) * BLK, CORE_NODES))
                nc.sync.dma_start(out=out_d[:, nsl],
                                  in_=h1_s[0:OUT_DIM, nsl])

    return nc


_PROGRAM_CACHE = {}


def kernel(**inputs) -> np.ndarray:
    import sys
    for p in ("/opt/trn_rl_repo",):
        if os.path.isdir(p) and p not in sys.path:
            sys.path.insert(0, p)
    from concourse.bass_utils import run_bass_kernel_spmd

    in_maps = make_inputs(inputs)  # sets geometry (TW) from the graph
    key = ("nc", TW)
    if key not in _PROGRAM_CACHE:
        nc = build_program()
        nc.finalize()
        _PROGRAM_CACHE[key] = nc
    nc = _PROGRAM_CACHE[key]
    res = run_bass_kernel_spmd(nc, in_maps, core_ids=list(range(NCORES)))
    vcol2 = np.array([(c % F2) * H2 + c // F2 for c in range(OUT_DIM)])
    outs = [res.results[c]["outT"].T[:, vcol2]  # un-permute v-order rows
            for c in range(NCORES)]
    return np.ascontiguousarray(np.concatenate(outs, 0)[:N_REAL]).astype(
        np.float32)


if __name__ == "__main__":
    import jax
    with jax.default_device(jax.devices("cpu")[0]):
        import reference
        inputs = {k: np.asarray(v) for k, v in reference.setup_inputs().items()}
        expected = np.asarray(reference.reference(**inputs))
    actual = kernel(**inputs)
    rel = np.linalg.norm(actual - expected) / np.linalg.norm(expected)
    print("Relative error:", rel)

